# revision 8
# baseline (speedup 1.0000x reference)
"""Trainium2 8-core kernel for batched attention + concat projection.

Reference computation (per batch b):
    scores = Q @ C^T                  [TQ, TC]
    A      = softmax(scores, axis=-1)
    mix    = A @ C                    [TQ, H]
    out    = tanh(concat([mix, Q]) @ W^T)   [TQ, H]

Distribution: pure data-parallel over batch (B=16 across 8 cores, 2
batches per core), W replicated. No collectives needed.

Layout strategy: Q^T, C^T are pre-transposed on the HOST (like W^T),
so the kernel does zero PE transposes for Q/C — Q^T/C^T DMA straight
into SBUF in the [feature, token] layout every matmul wants. C is
additionally sent in natural [token, feature] layout as bf16 for the
PV stage (lhsT), and W^T is sent pre-cast to bf16. The only on-device
transposes left are P^T (bf16, 1 cycle/row).

Per-core dataflow:
  - scores tile S[q,k] = qt.T @ ct  (fp32r matmuls, full speed at
    512-col blocks).
  - softmax over free axis k: DVE reduce_max(negate) -> ACT exp with
    per-partition bias + accumulated row sums -> DVE reciprocal ->
    DVE in-place scale (normalized P in bf16).
  - P^T via PE transposes (bf16, packed 8 per PSUM bank, drained by
    one wide ACT copy), mix^T = C.T @ P^T (bf16 matmuls, lhsT = C in
    natural [k, h] layout).
  - proj: out[q, :] = tanh(combT.T @ W^T) where combT = [mix^T; Q^T]
    in bf16.

The P^T/PV/proj stages for super-iteration s are emitted one
super-iteration later (software pipelining) so the in-order TensorE
stream always has matmul work while the softmax chain of the current
tile runs on ACT/DVE. The next batch's C^T/C DMAs are issued right
after the current batch's last QK so the transfer overlaps the
PT/PV/proj tail instead of stalling the next batch's head.
"""

import numpy as np
import ml_dtypes

import concourse.bacc as bacc
import concourse.tile as tile
import concourse.mybir as mybir
from concourse.bass_utils import run_bass_kernel_spmd

F32 = mybir.dt.float32
F32R = mybir.dt.float32r
BF16 = mybir.dt.bfloat16

N_CORES = 8
B, TQ, TC, H = 16, 2048, 2048, 1024


def build_bass(b_loc, tq, tc, h, n_cores=N_CORES):
    """Build the per-core Bass graph. All cores run the same graph (SPMD)."""
    d = 2 * h
    ho = h
    n_qt = tq // 128       # q tiles
    n_kt = tc // 128       # k tiles
    n_hc = h // 128        # h chunks
    n_dc = d // 128        # d chunks (contraction for proj)
    kb = min(512, tc)      # QK rhs block
    n_kb = tc // kb
    hob = min(512, ho)     # proj output block
    n_hob = ho // hob
    SUPER = 2              # q-tiles per super-iteration
    assert n_qt % SUPER == 0
    n_s = n_qt // SUPER
    sq = SUPER * 128       # q columns per super-iteration
    pg = min(8, n_kt)      # bf16 transposes packed per PSUM bank

    nc = bacc.Bacc("TRN2", target_bir_lowering=False, debug=False,
                   num_devices=n_cores)

    n_s_decl = tq // (2 * 128)
    qt_ext = nc.declare_dram_parameter(
        "qt", [b_loc, n_s_decl, 128, (h // 128) * 2 * 128], BF16,
        isOutput=False)
    ct_ext = nc.declare_dram_parameter("ct", [b_loc, h, tc], BF16, isOutput=False)
    cb_ext = nc.declare_dram_parameter("cb", [b_loc, tc, h], BF16, isOutput=False)
    wt_ext = nc.declare_dram_parameter("wtb", [d, ho], BF16, isOutput=False)
    idb_ext = nc.declare_dram_parameter("idb", [128, 128], BF16, isOutput=False)
    out_ext = nc.declare_dram_parameter("out", [b_loc, tq, ho], F32, isOutput=True)

    with tile.TileContext(nc) as tc_:
        with (
            tc_.tile_pool(name="const", bufs=1) as const_pool,
            tc_.tile_pool(name="ct", bufs=2) as ct_pool,
            tc_.tile_pool(name="cbf", bufs=1) as cbf_pool,
            tc_.tile_pool(name="qt", bufs=3) as qt_pool,
            tc_.tile_pool(name="p", bufs=2 * SUPER) as p_pool,
            tc_.tile_pool(name="ptb", bufs=1) as pt_pool,
            tc_.tile_pool(name="comb", bufs=2) as comb_pool,
            tc_.tile_pool(name="ostage", bufs=2) as out_pool,
            tc_.tile_pool(name="stats", bufs=12) as stats_pool,
            tc_.tile_pool(name="ps_s", bufs=1, space="PSUM") as ps_s,
            tc_.tile_pool(name="ps_tp", bufs=2, space="PSUM") as ps_tp,
            tc_.tile_pool(name="ps_mm", bufs=2, space="PSUM") as ps_mm,
        ):
            idb = const_pool.tile([128, 128], BF16, tag="idb")
            nc.sync.dma_start(idb[:], idb_ext[:])
            wt_bf = const_pool.tile([128, n_dc * ho], BF16, tag="wtbf")
            wt_r = wt_bf.rearrange("p (dc o) -> p dc o", o=ho)

            p_tiles = {}      # (b, t) -> normalized P tile
            combT_map = {}    # (b, s) -> combT tile
            pt_map = {}       # (b, s) -> P^T tile
            qt_map = {}       # (b, s) -> qt tile [128, n_hc * sq]

            def emit_qt_dma(b, s):
                qt_t = qt_pool.tile([128, n_hc * sq], BF16, tag="qt",
                                    name=f"qt_{b}_{s}")
                nc.sync.dma_start(qt_t[:], qt_ext[b, s])
                qt_map[(b, s)] = qt_t

            def emit_ct_dma(b):
                ct_all = ct_pool.tile([128, n_hc * tc], BF16, tag="ct",
                                      name=f"ct_{b}")
                ct_r = ct_all.rearrange("p (hc k) -> p hc k", k=tc)
                for kbi in range(n_kb):
                    for hc in range(n_hc):
                        nc.sync.dma_start(
                            ct_r[:, hc, kbi * kb:(kbi + 1) * kb],
                            ct_ext[b, hc * 128:(hc + 1) * 128,
                                   kbi * kb:(kbi + 1) * kb])
                return ct_all

            def emit_cb_dma(b):
                c_bf = cbf_pool.tile([128, n_kt * h], BF16, tag="cbf",
                                     name=f"cbf_{b}")
                c_r = c_bf.rearrange("p (kt hh) -> p kt hh", hh=h)
                for kt in range(n_kt):
                    nc.sync.dma_start(
                        c_r[:, kt, :],
                        cb_ext[b, kt * 128:(kt + 1) * 128, :])
                return c_bf

            def emit_wt_dma():
                for dc in range(n_dc):
                    nc.sync.dma_start(
                        wt_r[:, dc, :], wt_ext[dc * 128:(dc + 1) * 128, :])

            def emit_qk_softmax(b, s, ti, ct_all):
                """QK matmuls into one PSUM tile + wide softmax."""
                t = s * SUPER + ti
                qt_t = qt_map[(b, s)]
                qt_r = qt_t.rearrange("p (hc q) -> p hc q", q=sq)
                s_ps = ps_s.tile([128, tc], F32, tag="s", name=f"s_{b}_{t}")
                for kbi in range(n_kb):
                    for hc in range(n_hc):
                        lhs = qt_r[:, hc, ti * 128:(ti + 1) * 128]
                        rhs = ct_all[:, hc * tc + kbi * kb:
                                     hc * tc + (kbi + 1) * kb]
                        nc.tensor.matmul(
                            s_ps[:, kbi * kb:(kbi + 1) * kb], lhs, rhs,
                            start=(hc == 0), stop=(hc == n_hc - 1))

                negm = stats_pool.tile([128, 1], F32, tag="negm",
                                       name=f"negm_{b}_{t}")
                nc.vector.reduce_max(
                    negm[:], s_ps[:], axis=mybir.AxisListType.X, negate=True)
                l_tot = stats_pool.tile([128, 1], F32, tag="ltot",
                                        name=f"lt_{b}_{t}")
                nc.vector.memset(l_tot[:], 0.0)
                p = p_pool.tile([128, tc], BF16, tag="p", name=f"p_{b}_{t}")
                nc.scalar.activation(
                    p[:], s_ps[:], mybir.ActivationFunctionType.Exp,
                    bias=negm[:], scale=1.0, accum_out=l_tot[:])
                rcp = stats_pool.tile([128, 1], F32, tag="rcp",
                                      name=f"rcp_{b}_{t}")
                nc.vector.reciprocal(rcp[:], l_tot[:])
                nc.vector.tensor_scalar_mul(p[:], p[:], rcp[:])
                p_tiles[(b, t)] = p

            def emit_qcopy(b, s):
                """Copy Q^T (bf16 cast) into the comb tile's upper half."""
                combT = combT_map[(b, s)]
                comb_r = combT.rearrange("p (dc q) -> p dc q", q=sq)
                qt_t = qt_map.pop((b, s))
                qt_r = qt_t.rearrange("p (hc q) -> p hc q", q=sq)
                for ti in range(SUPER):
                    nc.vector.tensor_copy(
                        comb_r[:, n_hc:2 * n_hc, ti * 128:(ti + 1) * 128],
                        qt_r[:, :, ti * 128:(ti + 1) * 128])

            def emit_pt(b, s):
                """P^T for super s: PE transposes packed into PSUM banks,
                each drained by a single wide ACT copy."""
                pt_big = pt_pool.tile([128, n_kt * sq], BF16, tag="ptb",
                                      name=f"ptb_{b}_{s}")
                pt_r = pt_big.rearrange("p (k q) -> p k q", q=sq)
                for ti in range(SUPER):
                    p = p_tiles.pop((b, s * SUPER + ti))
                    for g in range(n_kt // pg):
                        tp8 = ps_tp.tile([128, pg * 128], BF16, tag="tp",
                                         name=f"tp8_{b}_{s}_{ti}_{g}")
                        for j in range(pg):
                            kt = pg * g + j
                            nc.tensor.transpose(
                                tp8[:, j * 128:(j + 1) * 128],
                                p[:, kt * 128:(kt + 1) * 128], idb[:])
                        nc.scalar.copy(
                            pt_r[:, pg * g: pg * (g + 1),
                                 ti * 128:(ti + 1) * 128],
                            tp8.rearrange("p (j c) -> p j c", c=128)[:])
                pt_map[(b, s)] = pt_big

            def emit_pv(b, s, c_bf):
                """PV matmuls: mix^T chunks into combT for super s."""
                combT = combT_map[(b, s)]
                pt_big = pt_map.pop((b, s))
                for hc in range(n_hc):
                    mm = ps_mm.tile([128, sq], F32, tag="mm",
                                    name=f"mm_{b}_{s}_{hc}")
                    for kt in range(n_kt):
                        nc.tensor.matmul(
                            mm[:],
                            c_bf[:, kt * h + hc * 128: kt * h + (hc + 1) * 128],
                            pt_big[:, kt * sq:(kt + 1) * sq],
                            start=(kt == 0), stop=(kt == n_kt - 1))
                    nc.vector.tensor_copy(
                        combT[:, hc * sq:(hc + 1) * sq], mm[:])

            def emit_proj(b, s):
                """Projection + tanh + store for both tiles of super s."""
                combT = combT_map.pop((b, s))
                for ti in range(SUPER):
                    t = s * SUPER + ti
                    ostage = out_pool.tile([128, ho], F32, tag="ostage",
                                           name=f"os_{b}_{t}")
                    for hb in range(n_hob):
                        pr = ps_mm.tile([128, hob], F32, tag="mm",
                                        name=f"pr_{b}_{t}_{hb}")
                        for dc in range(n_dc):
                            nc.tensor.matmul(
                                pr[:],
                                combT[:, dc * sq + ti * 128:
                                      dc * sq + (ti + 1) * 128],
                                wt_bf[:, dc * ho + hb * hob:
                                      dc * ho + (hb + 1) * hob],
                                start=(dc == 0), stop=(dc == n_dc - 1))
                        nc.scalar.activation(
                            ostage[:, hb * hob:(hb + 1) * hob], pr[:],
                            mybir.ActivationFunctionType.Tanh)
                    nc.sync.dma_start(
                        out_ext[b, t * 128:(t + 1) * 128, :], ostage[:])

            # batch 0 head: Q super 0, C^T, C, W
            emit_qt_dma(0, 0)
            ct_cur = emit_ct_dma(0)
            cb_cur = emit_cb_dma(0)
            emit_wt_dma()

            for b in range(b_loc):
                ct_all, c_bf = ct_cur, cb_cur
                for s in range(n_s):
                    if s + 1 < n_s:
                        emit_qt_dma(b, s + 1)
                    elif b + 1 < b_loc:
                        emit_qt_dma(b + 1, 0)
                    combT_map[(b, s)] = comb_pool.tile(
                        [128, n_dc * sq], BF16, tag="comb",
                        name=f"cb_{b}_{s}")
                    emit_qk_softmax(b, s, 0, ct_all)
                    if s > 0:
                        emit_pt(b, s - 1)
                    emit_qk_softmax(b, s, 1, ct_all)
                    emit_qcopy(b, s)
                    if s > 0:
                        emit_pv(b, s - 1, c_bf)
                        emit_proj(b, s - 1)
                    if s == 2 and b + 1 < b_loc:
                        ct_cur = emit_ct_dma(b + 1)
                # next batch's natural-layout C overlaps this batch's tail
                if b + 1 < b_loc:
                    cb_cur = emit_cb_dma(b + 1)
                emit_pt(b, n_s - 1)
                emit_pv(b, n_s - 1, c_bf)
                emit_proj(b, n_s - 1)

    nc.compile()
    return nc


_NC_CACHE = {}


def _get_nc(b_loc, tq, tc, h):
    key = (b_loc, tq, tc, h)
    if key not in _NC_CACHE:
        _NC_CACHE[key] = build_bass(b_loc, tq, tc, h)
    return _NC_CACHE[key]


def make_in_maps(query, context, W_attn, n_cores=N_CORES):
    b = query.shape[0]
    b_loc = b // n_cores
    wtb = np.ascontiguousarray(
        W_attn.T.astype(np.float32)).astype(ml_dtypes.bfloat16)
    idb = np.eye(128).astype(ml_dtypes.bfloat16)
    q32 = np.asarray(query, dtype=np.float32)
    c32 = np.asarray(context, dtype=np.float32)
    in_maps = []
    tq, h = q32.shape[1], q32.shape[2]
    n_s, sq, n_hc = tq // 256, 256, h // 128
    for i in range(n_cores):
        qs = q32[i * b_loc:(i + 1) * b_loc]
        cs = c32[i * b_loc:(i + 1) * b_loc]
        # [b, n_s, 128p, (hc sq)]: per-super Q^T blocks, one fat DMA each
        qth = np.ascontiguousarray(
            qs.reshape(b_loc, n_s, sq, n_hc, 128).transpose(0, 1, 4, 3, 2)
        ).reshape(b_loc, n_s, 128, n_hc * sq).astype(ml_dtypes.bfloat16)
        in_maps.append({
            "qt": qth,
            "ct": np.ascontiguousarray(cs.transpose(0, 2, 1)).astype(ml_dtypes.bfloat16),
            "cb": np.ascontiguousarray(cs).astype(ml_dtypes.bfloat16),
            "wtb": wtb,
            "idb": idb,
        })
    return in_maps


def kernel(query, context, W_attn, _trace=False, _trace_kwargs=None):
    b, tq, h = query.shape
    tc = context.shape[1]
    b_loc = b // N_CORES
    nc = _get_nc(b_loc, tq, tc, h)
    in_maps = make_in_maps(query, context, W_attn)
    res = run_bass_kernel_spmd(
        nc, in_maps, core_ids=list(range(N_CORES)), trace=_trace,
        **(_trace_kwargs or {}))
    out = np.concatenate([res.results[i]["out"] for i in range(N_CORES)], axis=0)
    if _trace:
        return out, res
    return out


# revision 9
# speedup vs baseline: 1.0265x; 1.0265x over previous
"""Trainium2 8-core kernel for batched attention + concat projection.

Reference computation (per batch b):
    scores = Q @ C^T                  [TQ, TC]
    A      = softmax(scores, axis=-1)
    mix    = A @ C                    [TQ, H]
    out    = tanh(concat([mix, Q]) @ W^T)   [TQ, H]

Distribution: pure data-parallel over batch (B=16 across 8 cores, 2
batches per core), W replicated. No collectives needed.

Layout strategy: Q^T, C^T are pre-transposed on the HOST (like W^T),
so the kernel does zero PE transposes for Q/C — Q^T/C^T DMA straight
into SBUF in the [feature, token] layout every matmul wants. C is
additionally sent in natural [token, feature] layout as bf16 for the
PV stage (lhsT), and W^T is sent pre-cast to bf16. The only on-device
transposes left are P^T (bf16, 1 cycle/row).

Per-core dataflow:
  - scores tile S[q,k] = qt.T @ ct  (fp32r matmuls, full speed at
    512-col blocks).
  - softmax over free axis k: DVE reduce_max(negate) -> ACT exp with
    per-partition bias + accumulated row sums -> DVE reciprocal ->
    DVE in-place scale (normalized P in bf16).
  - P^T via PE transposes (bf16, packed 8 per PSUM bank, drained by
    one wide ACT copy), mix^T = C.T @ P^T (bf16 matmuls, lhsT = C in
    natural [k, h] layout).
  - proj: out[q, :] = tanh(combT.T @ W^T) where combT = [mix^T; Q^T]
    in bf16.

The P^T/PV/proj stages for super-iteration s are emitted one
super-iteration later (software pipelining) so the in-order TensorE
stream always has matmul work while the softmax chain of the current
tile runs on ACT/DVE. The next batch's C^T/C DMAs are issued right
after the current batch's last QK so the transfer overlaps the
PT/PV/proj tail instead of stalling the next batch's head.
"""

import numpy as np
import ml_dtypes

import concourse.bacc as bacc
import concourse.tile as tile
import concourse.mybir as mybir
from concourse.bass_utils import run_bass_kernel_spmd

F32 = mybir.dt.float32
F32R = mybir.dt.float32r
BF16 = mybir.dt.bfloat16

N_CORES = 8
B, TQ, TC, H = 16, 2048, 2048, 1024


def build_bass(b_loc, tq, tc, h, n_cores=N_CORES):
    """Build the per-core Bass graph. All cores run the same graph (SPMD)."""
    d = 2 * h
    ho = h
    n_qt = tq // 128       # q tiles
    n_kt = tc // 128       # k tiles
    n_hc = h // 128        # h chunks
    n_dc = d // 128        # d chunks (contraction for proj)
    kb = min(512, tc)      # QK rhs block
    n_kb = tc // kb
    hob = min(512, ho)     # proj output block
    n_hob = ho // hob
    SUPER = 2              # q-tiles per super-iteration
    assert n_qt % SUPER == 0
    n_s = n_qt // SUPER
    sq = SUPER * 128       # q columns per super-iteration
    pg = min(8, n_kt)      # bf16 transposes packed per PSUM bank

    nc = bacc.Bacc("TRN2", target_bir_lowering=False, debug=False,
                   num_devices=n_cores)

    n_s_decl = tq // (2 * 128)
    qt_ext = nc.declare_dram_parameter(
        "qt", [b_loc, n_s_decl, 128, (h // 128) * 2 * 128], BF16,
        isOutput=False)
    ct_ext = nc.declare_dram_parameter("ct", [b_loc, h, tc], BF16, isOutput=False)
    cb_ext = nc.declare_dram_parameter("cb", [b_loc, tc, h], BF16, isOutput=False)
    wt_ext = nc.declare_dram_parameter("wtb", [d, ho], BF16, isOutput=False)
    idb_ext = nc.declare_dram_parameter("idb", [128, 128], BF16, isOutput=False)
    out_ext = nc.declare_dram_parameter("out", [b_loc, tq, ho], F32, isOutput=True)

    with tile.TileContext(nc) as tc_:
        with (
            tc_.tile_pool(name="const", bufs=1) as const_pool,
            tc_.tile_pool(name="ct", bufs=2) as ct_pool,
            tc_.tile_pool(name="cbf", bufs=1) as cbf_pool,
            tc_.tile_pool(name="qt", bufs=3) as qt_pool,
            tc_.tile_pool(name="p", bufs=2 * SUPER) as p_pool,
            tc_.tile_pool(name="ptb", bufs=1) as pt_pool,
            tc_.tile_pool(name="comb", bufs=2) as comb_pool,
            tc_.tile_pool(name="ostage", bufs=2) as out_pool,
            tc_.tile_pool(name="stats", bufs=12) as stats_pool,
            tc_.tile_pool(name="ps_s", bufs=1, space="PSUM") as ps_s,
            tc_.tile_pool(name="ps_tp", bufs=2, space="PSUM") as ps_tp,
            tc_.tile_pool(name="ps_mm", bufs=2, space="PSUM") as ps_mm,
        ):
            idb = const_pool.tile([128, 128], BF16, tag="idb")
            nc.sync.dma_start(idb[:], idb_ext[:])
            wt_bf = const_pool.tile([128, n_dc * ho], BF16, tag="wtbf")
            wt_r = wt_bf.rearrange("p (dc o) -> p dc o", o=ho)

            p_tiles = {}      # (b, t) -> normalized P tile
            combT_map = {}    # (b, s) -> combT tile
            pt_map = {}       # (b, s) -> P^T tile
            qt_map = {}       # (b, s) -> qt tile [128, n_hc * sq]

            def emit_qt_dma(b, s):
                qt_t = qt_pool.tile([128, n_hc * sq], BF16, tag="qt",
                                    name=f"qt_{b}_{s}")
                nc.sync.dma_start(qt_t[:], qt_ext[b, s])
                qt_map[(b, s)] = qt_t

            def emit_ct_dma(b):
                ct_all = ct_pool.tile([128, n_hc * tc], BF16, tag="ct",
                                      name=f"ct_{b}")
                ct_r = ct_all.rearrange("p (hc k) -> p hc k", k=tc)
                for hc in range(n_hc):
                    nc.sync.dma_start(
                        ct_r[:, hc, :],
                        ct_ext[b, hc * 128:(hc + 1) * 128, :])
                return ct_all

            def emit_cb_dma(b):
                c_bf = cbf_pool.tile([128, n_kt * h], BF16, tag="cbf",
                                     name=f"cbf_{b}")
                c_r = c_bf.rearrange("p (kt hh) -> p kt hh", hh=h)
                for kt in range(n_kt):
                    nc.sync.dma_start(
                        c_r[:, kt, :],
                        cb_ext[b, kt * 128:(kt + 1) * 128, :])
                return c_bf

            def emit_wt_dma():
                for dc in range(n_dc):
                    nc.sync.dma_start(
                        wt_r[:, dc, :], wt_ext[dc * 128:(dc + 1) * 128, :])

            def emit_qk_softmax(b, s, ti, ct_all):
                """QK matmuls into one PSUM tile + wide softmax."""
                t = s * SUPER + ti
                qt_t = qt_map[(b, s)]
                qt_r = qt_t.rearrange("p (hc q) -> p hc q", q=sq)
                s_ps = ps_s.tile([128, tc], F32, tag="s", name=f"s_{b}_{t}")
                for kbi in range(n_kb):
                    for hc in range(n_hc):
                        lhs = qt_r[:, hc, ti * 128:(ti + 1) * 128]
                        rhs = ct_all[:, hc * tc + kbi * kb:
                                     hc * tc + (kbi + 1) * kb]
                        nc.tensor.matmul(
                            s_ps[:, kbi * kb:(kbi + 1) * kb], lhs, rhs,
                            start=(hc == 0), stop=(hc == n_hc - 1))

                negm = stats_pool.tile([128, 1], F32, tag="negm",
                                       name=f"negm_{b}_{t}")
                nc.vector.reduce_max(
                    negm[:], s_ps[:], axis=mybir.AxisListType.X, negate=True)
                l_tot = stats_pool.tile([128, 1], F32, tag="ltot",
                                        name=f"lt_{b}_{t}")
                nc.vector.memset(l_tot[:], 0.0)
                p = p_pool.tile([128, tc], BF16, tag="p", name=f"p_{b}_{t}")
                nc.scalar.activation(
                    p[:], s_ps[:], mybir.ActivationFunctionType.Exp,
                    bias=negm[:], scale=1.0, accum_out=l_tot[:])
                rcp = stats_pool.tile([128, 1], F32, tag="rcp",
                                      name=f"rcp_{b}_{t}")
                nc.vector.reciprocal(rcp[:], l_tot[:])
                nc.vector.tensor_scalar_mul(p[:], p[:], rcp[:])
                p_tiles[(b, t)] = p

            def emit_qcopy(b, s):
                """Copy Q^T (bf16 cast) into the comb tile's upper half."""
                combT = combT_map[(b, s)]
                comb_r = combT.rearrange("p (dc q) -> p dc q", q=sq)
                qt_t = qt_map.pop((b, s))
                qt_r = qt_t.rearrange("p (hc q) -> p hc q", q=sq)
                for ti in range(SUPER):
                    nc.vector.tensor_copy(
                        comb_r[:, n_hc:2 * n_hc, ti * 128:(ti + 1) * 128],
                        qt_r[:, :, ti * 128:(ti + 1) * 128])

            def emit_pt(b, s):
                """P^T for super s: PE transposes packed into PSUM banks,
                each drained by a single wide ACT copy."""
                pt_big = pt_pool.tile([128, n_kt * sq], BF16, tag="ptb",
                                      name=f"ptb_{b}_{s}")
                pt_r = pt_big.rearrange("p (k q) -> p k q", q=sq)
                for ti in range(SUPER):
                    p = p_tiles.pop((b, s * SUPER + ti))
                    for g in range(n_kt // pg):
                        tp8 = ps_tp.tile([128, pg * 128], BF16, tag="tp",
                                         name=f"tp8_{b}_{s}_{ti}_{g}")
                        for j in range(pg):
                            kt = pg * g + j
                            nc.tensor.transpose(
                                tp8[:, j * 128:(j + 1) * 128],
                                p[:, kt * 128:(kt + 1) * 128], idb[:])
                        nc.scalar.copy(
                            pt_r[:, pg * g: pg * (g + 1),
                                 ti * 128:(ti + 1) * 128],
                            tp8.rearrange("p (j c) -> p j c", c=128)[:])
                pt_map[(b, s)] = pt_big

            def emit_pv(b, s, c_bf):
                """PV matmuls: mix^T chunks into combT for super s."""
                combT = combT_map[(b, s)]
                pt_big = pt_map.pop((b, s))
                for hc in range(n_hc):
                    mm = ps_mm.tile([128, sq], F32, tag="mm",
                                    name=f"mm_{b}_{s}_{hc}")
                    for kt in range(n_kt):
                        nc.tensor.matmul(
                            mm[:],
                            c_bf[:, kt * h + hc * 128: kt * h + (hc + 1) * 128],
                            pt_big[:, kt * sq:(kt + 1) * sq],
                            start=(kt == 0), stop=(kt == n_kt - 1))
                    nc.vector.tensor_copy(
                        combT[:, hc * sq:(hc + 1) * sq], mm[:])

            def emit_proj(b, s):
                """Projection + tanh + store for both tiles of super s."""
                combT = combT_map.pop((b, s))
                for ti in range(SUPER):
                    t = s * SUPER + ti
                    ostage = out_pool.tile([128, ho], F32, tag="ostage",
                                           name=f"os_{b}_{t}")
                    for hb in range(n_hob):
                        pr = ps_mm.tile([128, hob], F32, tag="mm",
                                        name=f"pr_{b}_{t}_{hb}")
                        for dc in range(n_dc):
                            nc.tensor.matmul(
                                pr[:],
                                combT[:, dc * sq + ti * 128:
                                      dc * sq + (ti + 1) * 128],
                                wt_bf[:, dc * ho + hb * hob:
                                      dc * ho + (hb + 1) * hob],
                                start=(dc == 0), stop=(dc == n_dc - 1))
                        nc.scalar.activation(
                            ostage[:, hb * hob:(hb + 1) * hob], pr[:],
                            mybir.ActivationFunctionType.Tanh)
                    nc.sync.dma_start(
                        out_ext[b, t * 128:(t + 1) * 128, :], ostage[:])

            # batch 0 head: Q super 0, C^T, C, W
            emit_qt_dma(0, 0)
            ct_cur = emit_ct_dma(0)
            cb_cur = emit_cb_dma(0)
            emit_wt_dma()

            for b in range(b_loc):
                ct_all, c_bf = ct_cur, cb_cur
                for s in range(n_s):
                    if s + 1 < n_s:
                        emit_qt_dma(b, s + 1)
                    elif b + 1 < b_loc:
                        emit_qt_dma(b + 1, 0)
                    combT_map[(b, s)] = comb_pool.tile(
                        [128, n_dc * sq], BF16, tag="comb",
                        name=f"cb_{b}_{s}")
                    emit_qk_softmax(b, s, 0, ct_all)
                    if s > 0:
                        emit_pt(b, s - 1)
                    emit_qk_softmax(b, s, 1, ct_all)
                    emit_qcopy(b, s)
                    if s > 0:
                        emit_pv(b, s - 1, c_bf)
                        emit_proj(b, s - 1)
                    if s == 2 and b + 1 < b_loc:
                        ct_cur = emit_ct_dma(b + 1)
                # next batch's natural-layout C overlaps this batch's tail
                if b + 1 < b_loc:
                    cb_cur = emit_cb_dma(b + 1)
                emit_pt(b, n_s - 1)
                emit_pv(b, n_s - 1, c_bf)
                emit_proj(b, n_s - 1)

    nc.compile()
    return nc


_NC_CACHE = {}


def _get_nc(b_loc, tq, tc, h):
    key = (b_loc, tq, tc, h)
    if key not in _NC_CACHE:
        _NC_CACHE[key] = build_bass(b_loc, tq, tc, h)
    return _NC_CACHE[key]


def make_in_maps(query, context, W_attn, n_cores=N_CORES):
    b = query.shape[0]
    b_loc = b // n_cores
    wtb = np.ascontiguousarray(
        W_attn.T.astype(np.float32)).astype(ml_dtypes.bfloat16)
    idb = np.eye(128).astype(ml_dtypes.bfloat16)
    q32 = np.asarray(query, dtype=np.float32)
    c32 = np.asarray(context, dtype=np.float32)
    in_maps = []
    tq, h = q32.shape[1], q32.shape[2]
    n_s, sq, n_hc = tq // 256, 256, h // 128
    for i in range(n_cores):
        qs = q32[i * b_loc:(i + 1) * b_loc]
        cs = c32[i * b_loc:(i + 1) * b_loc]
        # [b, n_s, 128p, (hc sq)]: per-super Q^T blocks, one fat DMA each
        qth = np.ascontiguousarray(
            qs.reshape(b_loc, n_s, sq, n_hc, 128).transpose(0, 1, 4, 3, 2)
        ).reshape(b_loc, n_s, 128, n_hc * sq).astype(ml_dtypes.bfloat16)
        in_maps.append({
            "qt": qth,
            "ct": np.ascontiguousarray(cs.transpose(0, 2, 1)).astype(ml_dtypes.bfloat16),
            "cb": np.ascontiguousarray(cs).astype(ml_dtypes.bfloat16),
            "wtb": wtb,
            "idb": idb,
        })
    return in_maps


def kernel(query, context, W_attn, _trace=False, _trace_kwargs=None):
    b, tq, h = query.shape
    tc = context.shape[1]
    b_loc = b // N_CORES
    nc = _get_nc(b_loc, tq, tc, h)
    in_maps = make_in_maps(query, context, W_attn)
    res = run_bass_kernel_spmd(
        nc, in_maps, core_ids=list(range(N_CORES)), trace=_trace,
        **(_trace_kwargs or {}))
    out = np.concatenate([res.results[i]["out"] for i in range(N_CORES)], axis=0)
    if _trace:
        return out, res
    return out
